# revision 1
# baseline (speedup 1.0000x reference)
"""Distributed single-head attention on 8 TRN2 NeuronCores.

Math (matches the reference):
    q = z @ Wq; k = z @ Wk; v = z @ Wv
    out = softmax(q k^T) * DK**-0.5 @ v

Sharding: z rows split 8 ways. Each core projects its own shard; K^T
(fp16) and V (bf16) shards are all-gathered in four async halves,
chained in consumption order kt1 -> vA -> kt2 -> vB on the serialized
CC stream. Flash-style row-block attention follows:
    S^T_j = K^T[:, j-tile] ^T-matmul Q^T           (fp16 operands, f32 PSUM)
    P_j   = exp(S^T_j - 40)                        (bf16, shift-invariant)
    rowsumT = ones^T @ P                           (PE)
    out   = (P^T-matmuls V) * (scale / rowsum)

Schedule notes (all measured on this part):
 - cross-core launch skew is ~25-30us and each AllGather costs
   ~22-30us wall on the ONE serialized CC stream, so the chain is
   co-critical with the PE. V is projected between the two K halves so
   the chain order becomes kt1, vA, kt2, vB (trigger FIFO on the
   gpsimd queue enforces it even though vB's data is staged early).
 - the AV phase is split by output half AND by j-parity into four
   segments: AV(h0, j%4<2) runs BETWEEN the S halves — it needs only
   S-half0's P tiles plus vA, and fills the PE stall while kt2 is in
   flight. Bank budget: 2 (S) + 1 (rowsum) + 4 (AV h0) = 7 of 8.
 - input HBM BW is ~250-300GB/s shared, so projections are t-outer
   across all 8 PSUM banks (first matmul needs only the first chunk
   pair); input loads are strictly need-ordered on the sync ring;
   staging DMAs ride the idle ACT ring; V loads ride SWDGE.
 - bounce buffers are (p, m, n)-contiguous per partition; each gather
   half is staged with ONE dma.

Precision: fp16 z/W/Q/K + f32 PSUM keeps logits to ~1e-2 abs err;
exp/V/AV in bf16. End-to-end rel err ~3e-3 (vs f32 reference).
"""

import numpy as np

SEQ, D, DK, DV = 4096, 1024, 1024, 1024
NCORES = 8
ROWS = SEQ // NCORES            # 512 rows per core
DT = D // 128                   # 8 contraction tiles (input dim)
MT = DK // 128                  # 8 dk tiles
ST = ROWS // 128                # 4 local seq tiles
JT = SEQ // 128                 # 32 global seq tiles
HN = ROWS // 2                  # 256 = half the local rows
SHIFT = 40.0                    # constant logit shift (softmax-invariant)
SCALE = DK ** -0.5

KT_H = DK * HN                  # elems in one K^T half (bf16-bitcast fp16)
V_H = HN * DV                   # elems in one V half (bf16)


def _build():
    import concourse.mybir as mybir
    import concourse.tile as tile
    from concourse import bacc

    F32 = mybir.dt.float32
    F16 = mybir.dt.float16
    BF16 = mybir.dt.bfloat16
    Exp = mybir.ActivationFunctionType.Exp

    nc = bacc.Bacc("TRN2", target_bir_lowering=False, debug=False, num_devices=NCORES)
    d_zT = nc.declare_dram_parameter("zT", [D, ROWS], F16, isOutput=False)
    d_wq = nc.declare_dram_parameter("Wq", [D, DK], F16, isOutput=False)
    d_wk = nc.declare_dram_parameter("Wk", [D, DK], F16, isOutput=False)
    d_wv = nc.declare_dram_parameter("Wv", [D, DV], F16, isOutput=False)
    d_out = nc.declare_dram_parameter("out", [ROWS, DV], F32, isOutput=True)

    rg = [list(range(NCORES))]

    with tile.TileContext(nc) as tc:
        with (
            tc.tile_pool(name="dram", bufs=1, space="DRAM") as dram,
            tc.tile_pool(name="qt", bufs=1) as qt_pool,
            tc.tile_pool(name="misc", bufs=1) as misc,
            tc.tile_pool(name="stage", bufs=2) as stage,
            tc.tile_pool(name="vg", bufs=1) as vg_pool,
            tc.tile_pool(name="expp", bufs=1) as expp,
            tc.tile_pool(name="outp", bufs=4) as outp,
        ):
            # ---- collective bounce buffers (per local-seq half) ----
            kt1_in = dram.tile([KT_H], BF16)
            kt1_out = dram.tile([NCORES * KT_H], BF16, addr_space="Shared")
            kt2_in = dram.tile([KT_H], BF16)
            kt2_out = dram.tile([NCORES * KT_H], BF16, addr_space="Shared")
            va_in = dram.tile([V_H], BF16)
            va_out = dram.tile([NCORES * V_H], BF16, addr_space="Shared")
            vb_in = dram.tile([V_H], BF16)
            vb_out = dram.tile([NCORES * V_H], BF16, addr_space="Shared")

            # constants: full-width ones for the PE rowsum (M=128 runs at
            # standard N=512 rate; M=1 measured ~40% slower), exp bias
            ones128 = misc.tile([128, 128], BF16)
            nc.vector.memset(ones128[:], 1.0)
            bias_sb = misc.tile([128, 1], F32)
            nc.vector.memset(bias_sb[:], -SHIFT)
            # touch Exp once so the ACT table set loads before the S phase
            warm_sb = misc.tile([128, 1], F32)
            nc.scalar.activation(warm_sb[:], bias_sb[:], Exp,
                                 bias=bias_sb[:], scale=1.0)

            # ---------------- projection phase (scoped weights) ----------
            with (
                tc.tile_pool(name="wz", bufs=1) as wz,
                tc.tile_pool(name="ps_proj", bufs=8, space="PSUM") as ps_proj,
            ):
                # zT + Wk chunk pairs interleaved on the sync HWDGE ring so
                # the t-outer K projection starts after one pair lands
                zv = d_zT.rearrange("(t p) n -> p t n", p=128)
                wkv = d_wk.rearrange("(t p) m -> p t m", p=128)
                zT_sb, wk_sb = [], []
                for t in range(DT):
                    zt = wz.tile([128, ROWS], F16, name=f"zt{t}")
                    nc.sync.dma_start(zt[:], zv[:, t, :])
                    zT_sb.append(zt)
                    w = wz.tile([128, DK], F16, name=f"wk{t}")
                    nc.sync.dma_start(w[:], wkv[:, t, :])
                    wk_sb.append(w)
                # Wv/Wq behind zT/Wk on the SAME sync ring: strict FIFO
                # defers them so the K-critical 3MB gets full HBM bandwidth
                wv_sb, wq_sb = [], []
                for d_w, prefix, tiles in ((d_wv, "wv", wv_sb), (d_wq, "wq", wq_sb)):
                    wvw = d_w.rearrange("(t p) m -> p t m", p=128)
                    for t in range(DT):
                        w = wz.tile([128, DK], F16, name=f"{prefix}{t}")
                        nc.sync.dma_start(w[:], wvw[:, t, :])
                        tiles.append(w)

                def k_half(half, kt_in, kt_out):
                    psk = [ps_proj.tile([128, HN], F32, tag="psproj",
                                        name=f"psk{half}{m}") for m in range(MT)]
                    for t in range(DT):
                        for m in range(MT):
                            nc.tensor.matmul(
                                psk[m][:], wk_sb[t][:, m * 128:(m + 1) * 128],
                                zT_sb[t][:, half * HN:(half + 1) * HN],
                                start=(t == 0), stop=(t == DT - 1))
                    kt_stage = stage.tile([128, MT, HN], F16, tag="ktstage")
                    for m in range(MT):
                        nc.vector.tensor_copy(kt_stage[:, m, :], psk[m][:])
                    nc.scalar.dma_start(
                        kt_in[:].rearrange("(p m n) -> p m n", p=128, n=HN),
                        kt_stage[:].bitcast(BF16))
                    nc.gpsimd.collective_compute(
                        "AllGather", mybir.AluOpType.bypass, replica_groups=rg,
                        ins=[kt_in[:].opt()], outs=[kt_out[:].opt()])

                # K^T first seq-half: earliest trigger on the CC chain
                k_half(0, kt1_in, kt1_out)

                # V next (not K half2): its vA gather is the SECOND link of
                # the CC chain, feeding the AV(h0) segment that runs
                # between the S halves
                psv = [ps_proj.tile([128, 512], F32, tag="psproj",
                                    name=f"psv{g}") for g in range(8)]
                for t in range(DT):
                    for s in range(ST):
                        for h in range(2):
                            nc.tensor.matmul(
                                psv[s * 2 + h][:],
                                zT_sb[t][:, s * 128:(s + 1) * 128],
                                wv_sb[t][:, h * 512:(h + 1) * 512],
                                start=(t == 0), stop=(t == DT - 1))
                v_stages = []
                for v_in, s0 in ((va_in, 0), (vb_in, 2)):
                    v_stage = stage.tile([128, 2, DV], BF16, tag="vstage")
                    for s in (s0, s0 + 1):
                        for h in range(2):
                            nc.vector.tensor_copy(
                                v_stage[:, s - s0, h * 512:(h + 1) * 512],
                                psv[s * 2 + h][:])
                    nc.scalar.dma_start(
                        v_in[:].rearrange("(p s m) -> p s m", p=128, m=DV),
                        v_stage[:])
                nc.gpsimd.collective_compute(
                    "AllGather", mybir.AluOpType.bypass, replica_groups=rg,
                    ins=[va_in[:].opt()], outs=[va_out[:].opt()])

                # K^T second seq-half, then vB: gpsimd trigger FIFO makes
                # the CC chain process kt2 before vB even though vB's data
                # was staged during the V projection
                k_half(1, kt2_in, kt2_out)
                nc.gpsimd.collective_compute(
                    "AllGather", mybir.AluOpType.bypass, replica_groups=rg,
                    ins=[vb_in[:].opt()], outs=[vb_out[:].opt()])

                # Q^T: [DK, ROWS] fp16 resident, t-outer (tolerates the
                # staggered wq arrival behind wv on the sync ring)
                qt_sb = qt_pool.tile([128, MT, ROWS], F16)
                psq = [ps_proj.tile([128, 512], F32, tag="psproj",
                                    name=f"psq{m}") for m in range(MT)]
                for t in range(DT):
                    for m in range(MT):
                        nc.tensor.matmul(
                            psq[m][:], wq_sb[t][:, m * 128:(m + 1) * 128],
                            zT_sb[t][:],
                            start=(t == 0), stop=(t == DT - 1))
                for m in range(MT):
                    nc.vector.tensor_copy(qt_sb[:, m, :], psq[m][:])

            # V gathered: resident [128, JT, DV] bf16 (64KB/partition).
            # SWDGE loads, one per rank per gather half, in (vA, vB) =
            # consumption order. Both alternatives measured worse: a single
            # 4D-AP DMA per half is slower and coarsens the dependency
            # (+25us stall); the sync ring lets the scheduler interleave
            # these ahead of the K^T loads (+35us).
            v_sb = vg_pool.tile([128, JT, DV], BF16)
            for v_out, soff in ((va_out, 0), (vb_out, 2)):
                for b in range(NCORES):
                    src = v_out[b * V_H:(b + 1) * V_H].rearrange(
                        "(p s m) -> p s m", p=128, m=DV)
                    nc.gpsimd.dma_start(
                        v_sb[:, b * ST + soff:b * ST + soff + 2, :], src)

            expS = expp.tile([128, JT, ROWS], BF16)

            # ---------------- S + AV phases -------------------------------
            # AV is split by output half h AND j-parity: (h0, j%4<2) runs
            # between the S halves; the rest after S. Each po group spans
            # both its passes (start on first p0 matmul, stop on last p1).
            j_pass = ([j for j in range(JT) if j % 4 < 2],
                      [j for j in range(JT) if j % 4 >= 2])

            def av_segment(po_h, h, part, rs=range(ST)):
                for r in rs:
                    p = po_h[r]
                    for idx, j in enumerate(j_pass[part]):
                        nc.tensor.matmul(
                            p[:],
                            expS[:, j, r * 128:(r + 1) * 128],
                            v_sb[:, j, h * 512:(h + 1) * 512],
                            start=(part == 0 and idx == 0),
                            stop=(part == 1 and idx == len(j_pass[1]) - 1))
                    if part == 1:
                        o_sb = outp.tile([128, 512], F32, tag="osb")
                        nc.vector.tensor_scalar_mul(o_sb[:], p[:],
                                                    mult_sb[:, r:r + 1])
                        nc.sync.dma_start(
                            d_out[r * 128:(r + 1) * 128,
                                  h * 512:(h + 1) * 512],
                            o_sb[:])

            with (
                tc.tile_pool(name="ps_oh0", bufs=4, space="PSUM") as ps_oh0,
                tc.tile_pool(name="ps_oh1a", bufs=1, space="PSUM") as ps_oh1a,
            ):
                po_h0 = [ps_oh0.tile([128, 512], F32, tag="poh0",
                                     name=f"poh0{r}") for r in range(ST)]
                # one spare bank: the (h1, r0) accumulator joins the
                # S-interleaved AV work so the filler segment (~21us)
                # matches the kt2 gather gap (~22us)
                po_h1 = [ps_oh1a.tile([128, 512], F32, tag="poh1a",
                                      name="poh1a0")]
                with (
                    tc.tile_pool(name="ktg", bufs=6) as ktg_pool,
                    tc.tile_pool(name="ps_s", bufs=2, space="PSUM") as ps_s,
                    tc.tile_pool(name="ps_rs", bufs=1, space="PSUM") as ps_rs,
                ):
                    rs_ps = ps_rs.tile([128, 512], F32)
                    n_rs = 0

                    def s_half(half, kt_out_h):
                        nonlocal n_rs
                        for b in range(NCORES):
                            ktb = ktg_pool.tile([128, MT, HN], F16, tag="ktg")
                            src = kt_out_h[b * KT_H:(b + 1) * KT_H].rearrange(
                                "(p m n) -> p m n", p=128, n=HN).bitcast(F16)
                            nc.sync.dma_start(ktb[:], src)
                            for jj in range(2):
                                j = b * ST + half * 2 + jj
                                ps_S = ps_s.tile([128, 512], F32, tag="pss")
                                for t in range(MT):
                                    nc.tensor.matmul(
                                        ps_S[:],
                                        ktb[:, t, jj * 128:(jj + 1) * 128],
                                        qt_sb[:, t, :],
                                        start=(t == 0), stop=(t == MT - 1))
                                nc.scalar.activation(expS[:, j, :], ps_S[:],
                                                     Exp, bias=bias_sb[:],
                                                     scale=1.0)
                                nc.tensor.matmul(rs_ps[:], ones128[:],
                                                 expS[:, j, :],
                                                 start=(n_rs == 0),
                                                 stop=(n_rs == JT - 1))
                                n_rs += 1

                    s_half(0, kt1_out)
                    # AV(h0 + h1r0, pass0): needs only S-half0's P tiles
                    # + vA; fills the PE while kt2 is still in flight
                    av_segment(po_h0, 0, 0)
                    av_segment(po_h1, 1, 0, rs=[0])
                    s_half(1, kt2_out)

                    # row-sum -> per-row reciprocal multipliers [128, ST]
                    rs_sb = misc.tile([1, 512], F32)
                    nc.vector.tensor_copy(rs_sb[:], rs_ps[0:1, :])
                    rs_dram = dram.tile([1, 512], F32)
                    nc.sync.dma_start(rs_dram[:], rs_sb[:])
                    rs128 = misc.tile([128, ST], F32)
                    nc.sync.dma_start(
                        rs128[:], rs_dram[0, :].rearrange("(r p) -> p r",
                                                          p=128))
                    mult_sb = misc.tile([128, ST], F32)
                    nc.vector.reciprocal(mult_sb[:], rs128[:])
                    nc.vector.tensor_scalar_mul(mult_sb[:], mult_sb[:], SCALE)

                with tc.tile_pool(name="ps_oh1", bufs=3, space="PSUM") as ps_oh1:
                    po_h1.extend(ps_oh1.tile([128, 512], F32, tag="poh1",
                                             name=f"poh1{r}")
                                 for r in range(1, ST))
                    av_segment(po_h1, 1, 0, rs=[1, 2, 3])
                    av_segment(po_h0, 0, 1)
                    av_segment(po_h1, 1, 1)
    nc.compile()
    return nc


_BUILT = None


def kernel(z, Wq, Wk, Wv):
    global _BUILT
    from concourse.bass_utils import run_bass_kernel_spmd

    if _BUILT is None:
        _BUILT = _build()
    nc = _BUILT

    zT = np.ascontiguousarray(z.T).astype(np.float16)
    wq16 = Wq.astype(np.float16)
    wk16 = Wk.astype(np.float16)
    wv16 = Wv.astype(np.float16)
    in_maps = [
        {
            "zT": np.ascontiguousarray(zT[:, c * ROWS:(c + 1) * ROWS]),
            "Wq": wq16,
            "Wk": wk16,
            "Wv": wv16,
        }
        for c in range(NCORES)
    ]
    res = run_bass_kernel_spmd(nc, in_maps, list(range(NCORES)))
    out = np.concatenate([res.results[c]["out"] for c in range(NCORES)], axis=0)
    return out.astype(np.float32)


if __name__ == "__main__":
    rng = np.random.default_rng(0)
    z = rng.standard_normal((SEQ, D)).astype(np.float32)
    Wq = (0.02 * rng.standard_normal((D, DK))).astype(np.float32)
    Wk = (0.02 * rng.standard_normal((D, DK))).astype(np.float32)
    Wv = (0.02 * rng.standard_normal((D, DV))).astype(np.float32)
    out = kernel(z=z, Wq=Wq, Wk=Wk, Wv=Wv)
    print(out.shape, out.dtype)



# revision 5
# speedup vs baseline: 1.4397x; 1.4397x over previous
"""Distributed single-head attention on 8 TRN2 NeuronCores — zero-collective.

Math (matches the reference):
    q = z @ Wq; k = z @ Wk; v = z @ Wv
    out = softmax(q k^T) * DK**-0.5 @ v

Key idea: every core receives the FULL z (inputs are full-size anyway), so
k and v never need to be materialized or all-gathered.  Using associativity:
    S_r  = q_r k^T = (z_r Wq) Wk^T z^T      -> B^T = Wk q_r^T, S^T = z B^T
    out_r = P_r v  = (P_r z) Wv             -> C^T = z^T-accum of P^T, out = C^T^T Wv
Per-core FLOPs are identical to the gather-based flash schedule (736 unit
matmuls), but there are NO collectives: no skew-absorbing barrier, no
serialized CC stream, no PE stall waiting for gathered K/V (the baseline
idled the PE 38us there, which also re-throttled the HAM clock gate).

Sharding: core c processes rows [512c, 512c+512).  Host ships z rolled so
each core's own block is first: zT_roll (d-major, for S) and zn_roll
(seq-major, for C).  A j-tile index in the kernel is the global row
(c*512 + 128j) mod 4096 — a pure permutation, harmless under the j-sums.

Phases (all PE-dense, back-to-back):
    q^T (64 MM) -> B^T (64) -> S^T/exp/rowsum (256+32) -> C^T (256) -> out (64)
DMA: ~22MB of params per core at ~265GB/s on the sync ring (zT, weights),
zn streamed on the gpsimd ring during the C pass, Wv + rowsum round-trip on
the scalar ring.  PSUM: 8 banks for projections, 2+1 for S/rowsum, 8 for
C^T, 8 for out — sequential scopes.

Precision: fp16 z/W/q/B + f32 PSUM keeps logits to ~6e-3 abs err; exp and
C^T in bf16 (range: logits can reach ~70 pre-shift, so exp(S-40) can hit
e^30 — fp16 would overflow, bf16 is safe).  End-to-end rel err ~3e-3.
"""

import numpy as np

SEQ, D, DK, DV = 4096, 1024, 1024, 1024
NCORES = 8
ROWS = SEQ // NCORES            # 512 rows per core
DT = D // 128                   # 8 contraction tiles (input dim)
MT = DK // 128                  # 8 dk tiles
ST = ROWS // 128                # 4 local seq tiles
JT = SEQ // 128                 # 32 global seq tiles
SHIFT = 40.0                    # constant logit shift (softmax-invariant)
SCALE = DK ** -0.5


def _build():
    import concourse.mybir as mybir
    import concourse.tile as tile
    from concourse import bacc

    F32 = mybir.dt.float32
    F16 = mybir.dt.float16
    BF16 = mybir.dt.bfloat16
    Exp = mybir.ActivationFunctionType.Exp

    nc = bacc.Bacc("TRN2", target_bir_lowering=False, debug=False, num_devices=NCORES)
    d_zT = nc.declare_dram_parameter("zT", [D, SEQ], F16, isOutput=False)
    d_zn = nc.declare_dram_parameter("zn", [SEQ, D], F16, isOutput=False)
    d_wq = nc.declare_dram_parameter("Wq", [D, DK], F16, isOutput=False)
    d_wkt = nc.declare_dram_parameter("WkT", [DK, D], F16, isOutput=False)
    d_wv = nc.declare_dram_parameter("Wv", [D, DV], F16, isOutput=False)
    d_out = nc.declare_dram_parameter("out", [ROWS, DV], F32, isOutput=True)

    with tile.TileContext(nc) as tc:
        with (
            tc.tile_pool(name="dram", bufs=1, space="DRAM") as dram,
            tc.tile_pool(name="misc", bufs=1) as misc,
            tc.tile_pool(name="zt", bufs=1) as ztp,
            tc.tile_pool(name="expp", bufs=1) as expp,
            tc.tile_pool(name="wvp", bufs=1) as wvp,
            tc.tile_pool(name="qb", bufs=1) as qbp,
            tc.tile_pool(name="outp", bufs=4) as outp,
        ):
            # constants: full-width ones for the PE rowsum, exp bias; touch
            # Exp once so the ACT table set loads before the S phase
            ones128 = misc.tile([128, 128], BF16)
            nc.vector.memset(ones128[:], 1.0)
            bias_sb = misc.tile([128, 1], F32)
            nc.vector.memset(bias_sb[:], -SHIFT)
            warm_sb = misc.tile([128, 1], F32)
            nc.scalar.activation(warm_sb[:], bias_sb[:], Exp,
                                 bias=bias_sb[:], scale=1.0)

            # ---- resident loads ------------------------------------------
            # sync ring, strict need order: (wq_t, zT jchunk0_t) pairs so the
            # t-outer q projection starts after one pair; then WkT for B^T;
            # then the rest of zT for the S phase.
            ztv = d_zT.rearrange("(t p) n -> p t n", p=128)
            wqv = d_wq.rearrange("(t p) m -> p t m", p=128)
            wktv = d_wkt.rearrange("(t p) m -> p t m", p=128)
            wvv = d_wv.rearrange("(t p) m -> p t m", p=128)

            zt_sb = ztp.tile([128, DT, SEQ], F16)
            # Wv rides the scalar ring (needed only at the out phase)
            wv_sb = wvp.tile([128, DT, DV], F16)
            for t in range(DT):
                nc.scalar.dma_start(wv_sb[:, t, :], wvv[:, t, :])

            B_sb = qbp.tile([128, MT, ROWS], F16)
            expS = expp.tile([128, JT, ROWS], BF16)

            with (
                tc.tile_pool(name="wqk", bufs=1) as wqk,
                tc.tile_pool(name="ps_proj", bufs=8, space="PSUM") as psp,
            ):
                wq_sb, wkt_sb = [], []
                for t in range(DT):
                    w = wqk.tile([128, DK], F16, name=f"wq{t}")
                    nc.sync.dma_start(w[:], wqv[:, t, :])
                    wq_sb.append(w)
                    nc.sync.dma_start(zt_sb[:, t, 0:1024], ztv[:, t, 0:1024])
                for t in range(DT):
                    w = wqk.tile([128, D], F16, name=f"wkt{t}")
                    nc.sync.dma_start(w[:], wktv[:, t, :])
                    wkt_sb.append(w)
                for c in range(1, 4):
                    for t in range(DT):
                        nc.sync.dma_start(zt_sb[:, t, c * 1024:(c + 1) * 1024],
                                          ztv[:, t, c * 1024:(c + 1) * 1024])

                # q^T[dk, r] = sum_t Wq[t-rows, dk-slice]^T zT[t-rows, own r]
                q_sb = wqk.tile([128, MT, ROWS], F16, name="qsb")
                psq = [psp.tile([128, ROWS], F32, tag="psp", name=f"psq{m}")
                       for m in range(MT)]
                for t in range(DT):
                    for m in range(MT):
                        nc.tensor.matmul(
                            psq[m][:], wq_sb[t][:, m * 128:(m + 1) * 128],
                            zt_sb[:, t, 0:ROWS],
                            start=(t == 0), stop=(t == DT - 1))
                # copies split across vector/scalar so the B phase's bank
                # reuse isn't gated on one engine draining 8 copies
                for m in range(MT):
                    eng = nc.vector.tensor_copy if m % 2 == 0 else nc.scalar.copy
                    eng(q_sb[:, m, :], psq[m][:])

                # B^T[d, r] = sum_dk Wk[d-slice, dk]^T q^T -> lhsT = WkT tiles
                psb = [psp.tile([128, ROWS], F32, tag="psp", name=f"psb{m}")
                       for m in range(MT)]
                for t in range(MT):
                    for m in range(DT):
                        nc.tensor.matmul(
                            psb[m][:], wkt_sb[t][:, m * 128:(m + 1) * 128],
                            q_sb[:, t, :],
                            start=(t == 0), stop=(t == MT - 1))
                for m in range(DT):
                    eng = nc.vector.tensor_copy if m % 2 == 0 else nc.scalar.copy
                    eng(B_sb[:, m, :], psb[m][:])

            # ---------------- S phase -------------------------------------
            # S^T[j, r] = sum_t zT[t, j-slice]^T B^T[t, r]; exp on ACT with
            # the -SHIFT bias; rowsum via ones-matmul accumulated across all
            # j into one persistent PSUM bank.
            # zn tiles for the C phase stream on the gpsimd ring; issue all
            # 32 up front — the 12-buf pool throttles the ring, which carries
            # nothing else.
            znv = d_zn.rearrange("(j p) m -> p j m", p=128)
            with (
                tc.tile_pool(name="znp", bufs=12) as znp,
                tc.tile_pool(name="csp", bufs=1) as csp,
            ):
                zn_sb = []
                for j in range(JT):
                    zn_t = znp.tile([128, D], F16, tag="zn", name=f"zn{j}")
                    nc.gpsimd.dma_start(zn_t[:], znv[:, j, :])
                    zn_sb.append(zn_t)

                mult_sb = misc.tile([128, ST], F32)
                with (
                    tc.tile_pool(name="ps_s", bufs=2, space="PSUM") as ps_s,
                    tc.tile_pool(name="ps_rs", bufs=1, space="PSUM") as ps_rs,
                ):
                    rs_ps = ps_rs.tile([128, ROWS], F32)
                    # rowsum for j is issued AFTER S j+1's matmuls: it waits
                    # on ACT's exp(j), which then overlaps S j+1 on the PE
                    def rowsum(j):
                        nc.tensor.matmul(rs_ps[:], ones128[:], expS[:, j, :],
                                         start=(j == 0), stop=(j == JT - 1))

                    for j in range(JT):
                        ps_S = ps_s.tile([128, ROWS], F32, tag="pss")
                        for t in range(DT):
                            nc.tensor.matmul(
                                ps_S[:], zt_sb[:, t, j * 128:(j + 1) * 128],
                                B_sb[:, t, :],
                                start=(t == 0), stop=(t == DT - 1))
                        nc.scalar.activation(expS[:, j, :], ps_S[:], Exp,
                                             bias=bias_sb[:], scale=1.0)
                        if j > 0:
                            rowsum(j - 1)
                    rowsum(JT - 1)

                    # row-sum -> per-row reciprocal multipliers [128, ST]
                    rs_sb = misc.tile([1, ROWS], F32)
                    nc.vector.tensor_copy(rs_sb[:], rs_ps[0:1, :])
                    rs_dram = dram.tile([1, ROWS], F32)
                    nc.scalar.dma_start(rs_dram[:], rs_sb[:])
                    rs128 = misc.tile([128, ST], F32)
                    nc.scalar.dma_start(
                        rs128[:], rs_dram[0, :].rearrange("(r p) -> p r",
                                                          p=128))
                    nc.vector.reciprocal(mult_sb[:], rs128[:])
                    nc.vector.tensor_scalar_mul(mult_sb[:], mult_sb[:], SCALE)

                # ---------------- C phase ---------------------------------
                # C^T[d, r] = sum_j zn[j, d-slice]^T P^T[j, r]
                C_sb = csp.tile([128, MT, ROWS], BF16)
                with tc.tile_pool(name="ps_c", bufs=8, space="PSUM") as ps_c:
                    psc = [ps_c.tile([128, ROWS], F32, tag="psc",
                                     name=f"psc{m}") for m in range(MT)]
                    for j in range(JT):
                        for m in range(MT):
                            nc.tensor.matmul(
                                psc[m][:], zn_sb[j][:, m * 128:(m + 1) * 128],
                                expS[:, j, :],
                                start=(j == 0), stop=(j == JT - 1))
                    for m in range(MT):
                        eng = (nc.vector.tensor_copy if m % 2 == 0
                               else nc.scalar.copy)
                        eng(C_sb[:, m, :], psc[m][:])

            # ---------------- out phase -----------------------------------
            # out[r-slice, e] = sum_m C^T[m, r-slice]^T Wv[m, e]
            with tc.tile_pool(name="ps_o", bufs=8, space="PSUM") as ps_o:
                for r in range(ST):
                    for h in range(2):
                        po = ps_o.tile([128, 512], F32, tag="po",
                                       name=f"po{r}{h}")
                        for m in range(MT):
                            nc.tensor.matmul(
                                po[:], C_sb[:, m, r * 128:(r + 1) * 128],
                                wv_sb[:, m, h * 512:(h + 1) * 512],
                                start=(m == 0), stop=(m == MT - 1))
                        o_sb = outp.tile([128, 512], F32, tag="osb")
                        nc.vector.tensor_scalar_mul(o_sb[:], po[:],
                                                    mult_sb[:, r:r + 1])
                        nc.sync.dma_start(
                            d_out[r * 128:(r + 1) * 128,
                                  h * 512:(h + 1) * 512],
                            o_sb[:])
    nc.compile()
    return nc


_BUILT = None


def make_in_maps(z, Wq, Wk, Wv):
    zT = np.ascontiguousarray(z.T).astype(np.float16)
    zn = z.astype(np.float16)
    wq16 = Wq.astype(np.float16)
    wkt16 = np.ascontiguousarray(Wk.T).astype(np.float16)
    wv16 = Wv.astype(np.float16)
    in_maps = []
    for c in range(NCORES):
        in_maps.append({
            "zT": np.ascontiguousarray(np.roll(zT, -c * ROWS, axis=1)),
            "zn": np.ascontiguousarray(np.roll(zn, -c * ROWS, axis=0)),
            "Wq": wq16,
            "WkT": wkt16,
            "Wv": wv16,
        })
    return in_maps


def kernel(z, Wq, Wk, Wv):
    global _BUILT
    from concourse.bass_utils import run_bass_kernel_spmd

    if _BUILT is None:
        _BUILT = _build()
    nc = _BUILT

    in_maps = make_in_maps(z, Wq, Wk, Wv)
    res = run_bass_kernel_spmd(nc, in_maps, list(range(NCORES)))
    out = np.concatenate([res.results[c]["out"] for c in range(NCORES)], axis=0)
    return out.astype(np.float32)


if __name__ == "__main__":
    rng = np.random.default_rng(0)
    z = rng.standard_normal((SEQ, D)).astype(np.float32)
    Wq = (0.02 * rng.standard_normal((D, DK))).astype(np.float32)
    Wk = (0.02 * rng.standard_normal((D, DK))).astype(np.float32)
    Wv = (0.02 * rng.standard_normal((D, DV))).astype(np.float32)
    out = kernel(z=z, Wq=Wq, Wk=Wk, Wv=Wv)
    print(out.shape, out.dtype)
